# revision 12
# baseline (speedup 1.0000x reference)
"""ConvNeXt MLP + parallel MoE-LoRA kernel for TRN2 (8-core data parallel).

Per-core computation (tokens sharded across cores, feature-on-partition layout):
  orig = GELU(x @ W1 + b1) @ W2 + b2                       (base MLP)
  h    = GELU(x @ w_down_all)                               (LoRA down, stacked)
  moe  = sum_i (h_i * prob_i) @ w_up_i                      (weighted up-proj)
  out  = orig + moe

All GEMMs keep features on the partition dim, tokens on the free dim; the host
ships x transposed ([DIM, NT] per core) and reassembles the transposed output.
Routing weights are expanded on the host to [ER, NT] (prob of the owning
expert replicated across its R lora dims), so the device does no routing work.

Token chunks are processed in pairs sharing each stationary weight load
(LDW w; MM chunkA; MM chunkB), and a post-compile pass deletes the redundant
second Ldweights of every pair — on HW the weight (re)load serializes with the
matmul stream, so halving the Ldweights count buys real PE issue time.
"""

import numpy as np
import ml_dtypes

import concourse.bass as bass
import concourse.mybir as mybir
import concourse.tile as tile
from concourse import bacc

F32 = mybir.dt.float32
F32R = mybir.dt.float32r
BF16 = mybir.dt.bfloat16

DIM, HID, E, R = 384, 1536, 3, 8
KT = DIM // 128    # 3  k-tiles of x / W1 rows
MT = HID // 128    # 12 hid tiles
ER = E * R         # 24 stacked lora dims
CH = 512           # token chunk (free dim per matmul)


def _ap_sig(arg):
    """Stable signature for an instruction Argument (weights AP)."""
    try:
        return repr(arg)
    except Exception:
        return None


def dedup_ldweights(nc):
    """Delete InstLdweights that reload the identical weights AP while only
    Matmult instructions ran on PE since the previous load.  Safe because the
    paired InstMatmult keeps the weights AP in its `ins` (dependency graph
    unchanged) and a dropped Ldweights carries no semaphore waits/updates."""
    removed = 0
    for blk in nc.m.functions[0].blocks:
        last_sig = None
        kept = []
        for inst in blk.instructions:
            nm = type(inst).__name__
            if nm == "InstLdweights":
                si = inst.sync_info
                clean = si is None or (not si.on_wait and not si.on_update)
                sig = _ap_sig(inst.ins[0])
                if clean and sig is not None and sig == last_sig:
                    removed += 1
                    continue
                last_sig = sig
            elif nm == "InstMatmult":
                if getattr(inst, "is_transpose", False):
                    last_sig = None
            else:
                eng = getattr(inst, "engine", None)
                if eng == mybir.EngineType.PE:
                    last_sig = None
            kept.append(inst)
        blk.instructions[:] = kept
    return removed


def build_nc(NT, mm="bf16", nrep=1, num_devices=8, act="gelu", dedup=True):
    """Build the bass program for one core's shard of NT tokens."""
    assert NT % 128 == 0
    wdt = F32R if mm == "fp32r" else BF16
    actf = (mybir.ActivationFunctionType.Gelu if act == "gelu"
            else mybir.ActivationFunctionType.Identity)

    nc = bacc.Bacc("TRN2", target_bir_lowering=False, debug=False,
                   num_devices=num_devices)

    xT = nc.dram_tensor("xT", [DIM, NT], wdt, kind="ExternalInput").ap()
    W1 = nc.dram_tensor("W1", [DIM, HID], wdt, kind="ExternalInput").ap()
    W2 = nc.dram_tensor("W2", [HID, DIM], wdt, kind="ExternalInput").ap()
    WDN = nc.dram_tensor("wdn", [DIM, ER], wdt, kind="ExternalInput").ap()
    WUP = nc.dram_tensor("wup", [ER, DIM], wdt, kind="ExternalInput").ap()
    B1 = nc.dram_tensor("b1c", [128, MT], F32, kind="ExternalInput").ap()
    B2 = nc.dram_tensor("b2c", [128, KT], F32, kind="ExternalInput").ap()
    PRX = nc.dram_tensor("prx", [ER, NT], wdt, kind="ExternalInput").ap()
    OUT = nc.dram_tensor("outT", [DIM, NT], F32, kind="ExternalOutput").ap()

    # token chunks, processed in pairs sharing stationary weights
    chunks = []
    off = 0
    while off < NT:
        w = min(CH, NT - off)
        chunks.append((off, w))
        off += w
    groups = [tuple(chunks[i:i + 2]) for i in range(0, len(chunks), 2)]
    if len(groups) >= 2 and len(groups[-1]) == 1:
        # merge the odd tail chunk into the last group (one more moving
        # stream per stationary load instead of a whole extra group)
        groups = groups[:-2] + [groups[-2] + groups[-1]]

    with tile.TileContext(nc) as tc:
        with (
            tc.tile_pool(name="const", bufs=1) as const,
            tc.tile_pool(name="xin", bufs=4) as xin,
            tc.tile_pool(name="hact", bufs=4) as hact,
            tc.tile_pool(name="lora", bufs=2) as lora,
            tc.tile_pool(name="lorw", bufs=4) as lorw,
            tc.tile_pool(name="outp", bufs=3) as outp,
            tc.tile_pool(name="ph", bufs=4, space="PSUM") as ph,
            tc.tile_pool(name="po", bufs=4, space="PSUM") as po,
        ):
            w1sb = const.tile([128, KT, HID], wdt)
            w2sb = const.tile([128, MT, DIM], wdt)
            wdnsb = const.tile([128, KT, ER], wdt)
            wupsb = const.tile([ER, DIM], wdt)
            b1sb = const.tile([128, MT], F32)
            b2sb = const.tile([128, KT], F32)
            prxsb = const.tile([ER, NT], wdt)

            # DRAM views with the k-tile dim folded out of the partition dim,
            # so a whole [128, KT, *] block moves in ONE dma (each dma_start
            # costs ~565ns SP issue + ~625ns HWDGE gen + ~900ns sem prop)
            xTk = xT.rearrange("(k p) t -> p k t", k=KT)
            W1k = W1.rearrange("(k p) h -> p k h", k=KT)
            W2k = W2.rearrange("(k p) d -> p k d", k=MT)
            WDNk = WDN.rearrange("(k p) e -> p k e", k=KT)

            def load_x(off, w):
                xsb = xin.tile([128, KT, CH], wdt, tag="x")
                nc.sync.dma_start(out=xsb[:, :, :w],
                                  in_=xTk[:, :, off:off + w])
                return xsb

            def load_first():
                # W1 first (first group's GEMM1 gates the whole pipeline),
                # split in halves so m0-5 can start while m6-11 loads
                H2 = HID // 2
                nc.sync.dma_start(out=w1sb[:, :, :H2], in_=W1k[:, :, :H2])
                xs0 = [load_x(off, w) for (off, w) in groups[0]]
                nc.sync.dma_start(out=w1sb[:, :, H2:], in_=W1k[:, :, H2:])
                nc.sync.dma_start(out=wdnsb, in_=WDNk)
                nc.sync.dma_start(out=b1sb, in_=B1)
                nc.sync.dma_start(out=prxsb, in_=PRX)
                return xs0

            def load_rest():
                # needed only by the first stage2, one group later
                nc.sync.dma_start(out=w2sb, in_=W2k)
                nc.sync.dma_start(out=wupsb, in_=WUP)
                nc.sync.dma_start(out=b2sb, in_=B2)

            def stage1_group(grp, xsbs):
                """GEMM1 + GELU + lora down for a pair of chunks; each
                stationary weight streams all chunks of the group."""
                n = len(grp)
                hsbs = [hact.tile([128, MT, CH], wdt, tag="h", name=f"hsb{ci}")
                        for ci in range(len(grp))]
                for m in range(MT):
                    psts = [ph.tile([128, CH], F32, tag="ph", name=f"pst{ci}")
                            for ci in range(len(grp))]
                    for k in range(KT):
                        for ci, (off, w) in enumerate(grp):
                            nc.tensor.matmul(
                                psts[ci][:, :w],
                                w1sb[:, k, m * 128:(m + 1) * 128],
                                xsbs[ci][:, k, :w],
                                start=(k == 0), stop=(k == KT - 1))
                    for ci, (off, w) in enumerate(grp):
                        nc.scalar.activation(
                            out=hsbs[ci][:, m, :w], in_=psts[ci][:, :w],
                            func=actf, bias=b1sb[:, m:m + 1], scale=1.0)
                # lora down as a 13th (24-wide) output tile
                psls = [ph.tile([128, CH], F32, tag="ph", name=f"psl{ci}")
                        for ci in range(len(grp))]
                for k in range(KT):
                    for ci, (off, w) in enumerate(grp):
                        nc.tensor.matmul(
                            psls[ci][:ER, :w], wdnsb[:, k, :],
                            xsbs[ci][:, k, :w],
                            start=(k == 0), stop=(k == KT - 1))
                hws = []
                for ci, (off, w) in enumerate(grp):
                    csl = slice(off, off + w)
                    hl = lora.tile([ER, CH], F32, tag="hl")
                    nc.scalar.activation(out=hl[:, :w], in_=psls[ci][:ER, :w],
                                         func=actf)
                    hw = lorw.tile([ER, CH], wdt, tag="hw")
                    nc.vector.tensor_tensor(out=hw[:, :w], in0=hl[:, :w],
                                            in1=prxsb[:, csl],
                                            op=mybir.AluOpType.mult)
                    hws.append(hw)
                return hsbs, hws

            OUTd = OUT.rearrange("(d p) t -> p d t", d=KT)

            def stage2_group(grp, hsbs, hws, last=False):
                """GEMM2 + accumulated lora up, bias, store.  The final group
                stores per output d-tile so the drain tail stays short; other
                groups fuse each chunk's store into one DMA."""
                osbs = [outp.tile([128, KT, CH], F32, tag="o", name=f"osb{ci}")
                        for ci in range(len(grp))]
                for d in range(KT):
                    psos = [po.tile([128, CH], F32, tag="po", name=f"pso{ci}")
                            for ci in range(len(grp))]
                    for k in range(MT):
                        for ci, (off, w) in enumerate(grp):
                            nc.tensor.matmul(
                                psos[ci][:, :w],
                                w2sb[:, k, d * 128:(d + 1) * 128],
                                hsbs[ci][:, k, :w],
                                start=(k == 0), stop=False)
                    for ci, (off, w) in enumerate(grp):
                        nc.tensor.matmul(
                            psos[ci][:, :w], wupsb[:, d * 128:(d + 1) * 128],
                            hws[ci][:, :w], start=False, stop=True)
                    for ci, (off, w) in enumerate(grp):
                        nc.vector.tensor_scalar(
                            out=osbs[ci][:, d, :w], in0=psos[ci][:, :w],
                            scalar1=b2sb[:, d:d + 1], scalar2=None,
                            op0=mybir.AluOpType.add)
                        if last:
                            nc.sync.dma_start(
                                out=OUT[d * 128:(d + 1) * 128, off:off + w],
                                in_=osbs[ci][:, d, :w])
                if not last:
                    for ci, (off, w) in enumerate(grp):
                        nc.sync.dma_start(out=OUTd[:, :, off:off + w],
                                          in_=osbs[ci][:, :, :w])

            def body(_iv=None):
                xs = load_first()
                prev = None
                for gi, grp in enumerate(groups):
                    if gi + 1 < len(groups):
                        xs_next = [load_x(off, w) for (off, w) in groups[gi + 1]]
                    cur = (grp,) + stage1_group(grp, xs)
                    if gi == 0:
                        load_rest()
                    if prev is not None:
                        stage2_group(*prev)
                    prev = cur
                    xs = xs_next if gi + 1 < len(groups) else None
                stage2_group(*prev, last=True)

            if nrep == 1:
                body()
            else:
                with tc.For_i(0, nrep, 1,
                              hint_engines=(mybir.EngineType.PE,
                                            mybir.EngineType.Activation,
                                            mybir.EngineType.DVE,
                                            mybir.EngineType.SP)):
                    body()

    nc.compile()
    if dedup:
        dedup_ldweights(nc)
    return nc


# ---------------- host-side helpers ----------------

def shard_inputs(x, topk_probs, topk_idx, w_down, w_up, W1, b1, W2, b2,
                 n_cores=8, mm="bf16", scaling=1.0):
    """Full inputs -> list of per-core in_maps (plus NT per core)."""
    npdt = np.float32 if mm == "fp32r" else ml_dtypes.bfloat16
    x_flat = np.asarray(x, np.float32).reshape(-1, DIM)
    N = x_flat.shape[0]
    assert N % (n_cores * 128) == 0
    NT = N // n_cores

    W1h = np.ascontiguousarray(np.asarray(W1, np.float32)).astype(npdt)
    W2h = np.ascontiguousarray(np.asarray(W2, np.float32)).astype(npdt)
    wdn = np.concatenate([np.asarray(w_down[i], np.float32) for i in range(E)],
                         axis=1).astype(npdt)                       # [DIM, ER]
    wup = (np.concatenate([np.asarray(w_up[i], np.float32) for i in range(E)],
                          axis=0) * scaling).astype(npdt)           # [ER, DIM]
    b1c = np.ascontiguousarray(np.asarray(b1, np.float32).reshape(MT, 128).T)
    b2c = np.ascontiguousarray(np.asarray(b2, np.float32).reshape(KT, 128).T)

    # expanded routing weights: prx[e*R + r, t] = prob of expert e at token t
    idx = np.asarray(topk_idx)
    prb = np.asarray(topk_probs, np.float32)
    probE = np.zeros((E, N), np.float32)
    for i in range(E):
        probE[i] = np.where(idx == i, prb, 0.0).sum(axis=1)
    prx_full = np.repeat(probE, R, axis=0)                          # [ER, N]

    in_maps = []
    for c in range(n_cores):
        sl = slice(c * NT, (c + 1) * NT)
        xTc = np.ascontiguousarray(x_flat[sl].T).astype(npdt)
        prxc = np.ascontiguousarray(prx_full[:, sl]).astype(npdt)
        in_maps.append({
            "xT": xTc, "W1": W1h, "W2": W2h, "wdn": wdn, "wup": wup,
            "b1c": b1c, "b2c": b2c, "prx": prxc,
        })
    return in_maps, NT


def unshard_output(results, x_shape):
    outs = [r["outT"] for r in results]          # each [DIM, NT] f32
    full = np.concatenate(outs, axis=1)          # [DIM, N]
    return np.ascontiguousarray(full.T).reshape(x_shape)


# ---------------- self-contained entry point ----------------

_NC_CACHE = {}


def _get_nc(NT, mm="bf16", nrep=1):
    key = (NT, mm, nrep)
    if key not in _NC_CACHE:
        _NC_CACHE[key] = build_nc(NT, mm=mm, nrep=nrep, num_devices=8,
                                  act="gelu")
    return _NC_CACHE[key]


def kernel(x, gate, topk_probs, topk_idx, w_down, w_up, W1, b1, W2, b2):
    """Full (unsharded) inputs -> full output, 8-core data parallel over
    tokens.  `gate` is unused (the reference never reads it)."""
    from concourse.bass_utils import run_bass_kernel_spmd

    x = np.asarray(x)
    in_maps, NT = shard_inputs(
        x, np.asarray(topk_probs), np.asarray(topk_idx), np.asarray(w_down),
        np.asarray(w_up), np.asarray(W1), np.asarray(b1), np.asarray(W2),
        np.asarray(b2), n_cores=8, mm="bf16", scaling=8.0 / 8.0)
    nc = _get_nc(NT, mm="bf16", nrep=1)
    res = run_bass_kernel_spmd(nc, in_maps, core_ids=list(range(8)))
    return unshard_output(res.results, x.shape).astype(np.float32)
